# revision 28
# baseline (speedup 1.0000x reference)
"""Trainium2 Bass kernel for nn_EngramModule (hash-ngram embedding gating).

Sharding: data-parallel over batch across 8 NeuronCores (one batch row per
core); hash tables and weights replicated per core.

Per-core pipeline:
  - exact n-gram hashes on DVE in fp32 (fused multiply / round-to-nearest /
    two-term subtract modular reduction; no integer mod on the DVE ISA)
  - hash tensor PE-transposed so gather indices are plain column slices
  - 256 indirect DMAs gather embedding rows (128 rows of 256B each)
  - e PE-transposed to m-major; k = e @ key_w^T and v^T = value_w^T @ e^T
    run as bf16 matmuls (full PE rate), fp32 PSUM accumulate
  - per-token scalars: free-dim reductions (k side), PE ones-matmul (v side)
  - dilated causal depthwise conv: 4 shifted MACs along the free token axis
    in [d, t] layout with per-partition channel weights
  - out = silu(conv) + gate*v assembled by accumulating two PE transposes
    per block in PSUM, escaping fp32 to [t, d]
"""
from contextlib import ExitStack

import numpy as np
import ml_dtypes

import concourse.bass as bass
import concourse.mybir as mybir
import concourse.tile as tile
from concourse import bacc, bass_utils
from concourse.masks import make_identity

F32 = mybir.dt.float32
BF16 = mybir.dt.bfloat16
I32 = mybir.dt.int32
Alu = mybir.AluOpType
Act = mybir.ActivationFunctionType
AxX = mybir.AxisListType.X

# ---- module config (mirrors the reference) ----
MAX_NGRAM = 3
N_HEADS = 8
EMBED_DIM = 64
N_EMBD = 2048
KERNEL = 4
DILATION = MAX_NGRAM
VOCAB = 32768
BASE_TABLE = 3 * VOCAB
B, T = 8, 2048
D_MEM = (MAX_NGRAM - 1) * N_HEADS * EMBED_DIM  # 1024
EPS = float(np.finfo(np.float32).eps)


def _next_prime(n):
    if n <= 2:
        return 2
    if n % 2 == 0:
        n += 1
    while True:
        if all(n % d for d in range(3, int(n ** 0.5) + 1, 2)):
            return n
        n += 2


PRIMES, POWERS, NS = [], [], []
for _n in range(2, MAX_NGRAM + 1):
    for _k in range(N_HEADS):
        _p = _next_prime(BASE_TABLE + _n * _k * 997)
        _seed = _n * 2654435761 + _k * 40503
        PRIMES.append(_p)
        POWERS.append([pow(_seed, _n - _j, _p) for _j in range(_n)])
        NS.append(_n)
OFFSETS = np.concatenate([[0], np.cumsum(PRIMES)]).astype(np.int64)
TOTAL_ROWS = int(OFFSETS[-1])
NT = 16

# unified 3-tap hash weights; tap j multiplies ids[tau - (2 - j)]
W_UNI = []
for _i in range(NT):
    if NS[_i] == 2:
        W_UNI.append([0, POWERS[_i][0], POWERS[_i][1]])
    else:
        W_UNI.append(list(POWERS[_i]))

MAGIC = float(3 * 2 ** 22)
N_MEGA = 4
MEGA_T = T // N_MEGA
TILES_PER_MEGA = MEGA_T // 128   # 8
N_TILES = T // 128               # 16
N_DCH = N_EMBD // 128            # 16
TC_PER_MEGA = MEGA_T // 512      # 2
PAD = DILATION * (KERNEL - 1)    # 9


def _host_constants():
    part_i = np.arange(128) // 8

    def col(vals):
        return np.asarray(vals, np.float64)[part_i].astype(np.float32)

    order = []
    cols = []

    def add(name, vals):
        order.append(name)
        cols.append(col(vals))

    for j in range(3):
        add(f"a{j}", [W_UNI[i][j] >> 8 for i in range(NT)])
    for j in range(3):
        add(f"b{j}", [W_UNI[i][j] & 255 for i in range(NT)])
    add("invp", [np.float32(1.0) / np.float32(PRIMES[i]) for i in range(NT)])
    add("ph256", [(PRIMES[i] >> 8) * 256 for i in range(NT)])
    add("pl", [PRIMES[i] & 255 for i in range(NT)])
    add("p", [PRIMES[i] for i in range(NT)])
    add("negp", [-PRIMES[i] for i in range(NT)])
    add("off", [int(OFFSETS[i]) for i in range(NT)])
    arr = np.stack(cols, axis=1)
    return arr, {k: j for j, k in enumerate(order)}


CONSTS_PP, CIDX = _host_constants()


DEBUG_TAPS = False


def build_program():
    nc = bacc.Bacc("TRN2", target_bir_lowering=False, debug=False)

    sj_d = nc.dram_tensor("sj_all", [3, 128, 256], I32,
                          kind="ExternalInput").ap()
    tab_d = nc.dram_tensor("tables", [TOTAL_ROWS, EMBED_DIM], F32,
                           kind="ExternalInput").ap()
    x_d = nc.dram_tensor("xrow", [T, N_EMBD], F32, kind="ExternalInput").ap()
    wk_d = nc.dram_tensor("wkT", [D_MEM, N_EMBD], BF16,
                          kind="ExternalInput").ap()
    wv_d = nc.dram_tensor("wvT", [D_MEM, N_EMBD], BF16,
                          kind="ExternalInput").ap()
    cpp_d = nc.dram_tensor("consts_pp", [128, CONSTS_PP.shape[1]], F32,
                           kind="ExternalInput").ap()
    cw_d = nc.dram_tensor("conv_pp", [128, N_DCH * KERNEL], F32,
                          kind="ExternalInput").ap()
    out_d = nc.dram_tensor("out", [T, N_EMBD], F32, kind="ExternalOutput").ap()
    if DEBUG_TAPS:
        dbg_idx_d = nc.dram_tensor("dbg_idx", [2, 128, 128], I32,
                                   kind="ExternalOutput").ap()
        dbg_e_d = nc.dram_tensor("dbg_e", [128, NT * EMBED_DIM], F32,
                                 kind="ExternalOutput").ap()
        dbg_sc_d = nc.dram_tensor("dbg_sc", [8, 128, TILES_PER_MEGA], F32,
                                  kind="ExternalOutput").ap()
        dbg_vt_d = nc.dram_tensor("dbg_vt", [128, MEGA_T], F32,
                                  kind="ExternalOutput").ap()
        dbg_nv_d = nc.dram_tensor("dbg_nv", [128, PAD + MEGA_T], F32,
                                  kind="ExternalOutput").ap()
        dbg_cv_d = nc.dram_tensor("dbg_cv", [128, MEGA_T], F32,
                                  kind="ExternalOutput").ap()
        dbg_sg_d = nc.dram_tensor("dbg_sg", [128, MEGA_T], F32,
                                  kind="ExternalOutput").ap()
        dbg_rep_d = nc.dram_tensor("dbg_rep", [2, 128, MEGA_T], F32,
                                   kind="ExternalOutput").ap()

    with tile.TileContext(nc) as tc, ExitStack() as ctx:
        cpool = ctx.enter_context(tc.tile_pool(name="cpool", bufs=1))
        ps_tr = ctx.enter_context(tc.tile_pool(name="ps_tr", bufs=2,
                                               space="PSUM"))
        ps_k = ctx.enter_context(tc.tile_pool(name="ps_k", bufs=2,
                                              space="PSUM"))
        ps_v = ctx.enter_context(tc.tile_pool(name="ps_v", bufs=1,
                                              space="PSUM"))
        ps_s = ctx.enter_context(tc.tile_pool(name="ps_s", bufs=1,
                                              space="PSUM"))
        ps_o = ctx.enter_context(tc.tile_pool(name="ps_o", bufs=2,
                                              space="PSUM"))

        scrp = None

        def scr(tag, shape=(128, 256), dtype=F32, bufs=1, pool=None):
            return (pool or scrp).tile(list(shape), dtype, tag=tag, name=tag,
                                       bufs=bufs)

        def trps(shape=(128, 128)):
            return ps_tr.tile(list(shape), F32, space="PSUM", tag="trp",
                              name="trp", bufs=2)

        # ---------- constants ----------
        wk_sb, wv_sb = [], []
        for q in range(8):
            wkq = cpool.tile([128, N_EMBD], BF16, name=f"wk{q}")
            nc.sync.dma_start(wkq[:], wk_d[q * 128:(q + 1) * 128, :])
            wk_sb.append(wkq)
            wvq = cpool.tile([128, N_EMBD], BF16, name=f"wv{q}")
            nc.sync.dma_start(wvq[:], wv_d[q * 128:(q + 1) * 128, :])
            wv_sb.append(wvq)
        cpp = cpool.tile([128, CONSTS_PP.shape[1]], F32, name="cpp")
        nc.sync.dma_start(cpp[:], cpp_d[:])
        cw = cpool.tile([128, N_DCH * KERNEL], F32, name="cw")
        nc.sync.dma_start(cw[:], cw_d[:])
        ident_f = cpool.tile([128, 128], F32, name="ident_f")
        make_identity(nc, ident_f[:])
        ident_b = cpool.tile([128, 128], BF16, name="ident_b")
        nc.vector.tensor_copy(ident_b[:], ident_f[:])
        ones_col = cpool.tile([128, 1], BF16, name="ones_col")
        nc.vector.memset(ones_col[:], 1.0)
        ones_row = cpool.tile([1, 128], BF16, name="ones_row")
        nc.vector.memset(ones_row[:], 1.0)
        ones8 = cpool.tile([128, 8], BF16, name="ones8")
        nc.vector.memset(ones8[:], 1.0)

        def cc(name):
            return cpp[:, CIDX[name]:CIDX[name] + 1]

        # ---------- hash ----------
        hctx = ExitStack()
        hpool = hctx.enter_context(tc.tile_pool(name="hpool", bufs=1))
        s_f = []
        for j in range(3):
            sj_i = hpool.tile([128, 256], I32, name=f"s{j}_i")
            nc.sync.dma_start(sj_i[:], sj_d[j])
            sj_f = hpool.tile([128, 256], F32, name=f"s{j}_f")
            nc.vector.tensor_copy(sj_f[:], sj_i[:])
            s_f.append(sj_f)

        def reduce4(x):
            q = scr("hq", pool=hpool)
            nc.vector.tensor_scalar(q[:], x[:], cc("invp"), None, Alu.mult)
            nc.vector.tensor_scalar(q[:], q[:], MAGIC, MAGIC, Alu.add,
                                    Alu.subtract)
            r = scr("hr" + x.name[-2:], bufs=1, pool=hpool)
            nc.vector.scalar_tensor_tensor(r[:], q[:], cc("ph256"), x[:],
                                           Alu.mult, Alu.subtract)
            nc.vector.scalar_tensor_tensor(r[:], q[:], cc("pl"), r[:],
                                           Alu.mult, Alu.add)
            return r

        m1s, los = [], []
        for j in range(3):
            t_j = scr(f"ht{j}", pool=hpool)
            nc.vector.tensor_scalar(t_j[:], s_f[j][:], cc(f"a{j}"), None,
                                    Alu.mult)
            m1s.append(reduce4(t_j))
            lo = scr(f"hlo{j}", pool=hpool)
            nc.vector.tensor_scalar(lo[:], s_f[j][:], cc(f"b{j}"), None,
                                    Alu.mult)
            los.append(lo)
        U = scr("hU", pool=hpool)
        nc.vector.tensor_tensor(U[:], m1s[0][:], m1s[1][:], Alu.add)
        nc.vector.tensor_tensor(U[:], U[:], m1s[2][:], Alu.add)
        gsc = scr("hgs", pool=hpool)
        nc.vector.tensor_scalar(gsc[:], U[:], -256.0, None, Alu.mult)
        m2 = reduce4(gsc)
        L2 = scr("hL2", pool=hpool)
        nc.vector.tensor_tensor(L2[:], los[0][:], los[1][:], Alu.add)
        Lr = reduce4(L2)
        Hh = scr("hHa", pool=hpool)
        nc.vector.tensor_tensor(Hh[:], m2[:], Lr[:], Alu.add)
        nc.vector.tensor_tensor(Hh[:], Hh[:], los[2][:], Alu.subtract)
        Fh = reduce4(Hh)
        msk = scr("hmk", pool=hpool)
        for _ in range(2):
            nc.vector.tensor_scalar(msk[:], Fh[:], 0.0, None, Alu.is_lt)
            nc.vector.scalar_tensor_tensor(msk[:], msk[:], cc("p"), Fh[:],
                                           Alu.mult, Alu.add)
            Fh, msk = msk, Fh
        nc.vector.tensor_scalar(msk[:], Fh[:], cc("p"), None, Alu.is_ge)
        nc.vector.scalar_tensor_tensor(msk[:], msk[:], cc("negp"), Fh[:],
                                       Alu.mult, Alu.add)
        Fh, msk = msk, Fh
        nc.vector.tensor_scalar(Fh[:], Fh[:], cc("off"), None, Alu.add)

        hT_i32 = []
        for h in range(2):
            hT_ps = trps()
            nc.tensor.transpose(hT_ps[:], Fh[:, h * 128:(h + 1) * 128],
                                ident_f[:])
            hTi = cpool.tile([128, 128], I32, name=f"hTi{h}")
            nc.vector.tensor_copy(hTi[:], hT_ps[:])
            hT_i32.append(hTi)

        if DEBUG_TAPS:
            for h in range(2):
                nc.sync.dma_start(dbg_idx_d[h], hT_i32[h][:])
        hctx.close()

        scrp = ctx.enter_context(tc.tile_pool(name="scrp", bufs=1))
        epool = ctx.enter_context(tc.tile_pool(name="epool", bufs=4))
        etm_p = ctx.enter_context(tc.tile_pool(name="etm_p", bufs=2))
        xpool = ctx.enter_context(tc.tile_pool(name="xpool", bufs=3))
        vpool = ctx.enter_context(tc.tile_pool(name="vpool", bufs=16))
        dwork = ctx.enter_context(tc.tile_pool(name="dwork", bufs=2))
        spool = ctx.enter_context(tc.tile_pool(name="spool", bufs=2))
        obuf = ctx.enter_context(tc.tile_pool(name="obuf", bufs=4))
        tailp = ctx.enter_context(tc.tile_pool(name="tailp", bufs=1))

        def idx_col(jt, i):
            h, g = jt % 2, jt // 2
            return hT_i32[h][:, i * 8 + g: i * 8 + g + 1]

        # ---------- per-d-chunk conv tails ----------
        tails = []
        for c in range(N_DCH):
            tl = tailp.tile([128, PAD], BF16, name=f"tail{c}")
            nc.vector.memset(tl[:], 0.0)
            tails.append(tl)

        for mega in range(N_MEGA):
            eT_mega = etm_p.tile([128, 8 * MEGA_T], BF16, tag="eTm",
                                 name="eT_mega", bufs=2)
            sxk_t = spool.tile([128, TILES_PER_MEGA], F32, tag="sxk",
                               name="sxk_t")
            sk2_t = spool.tile([128, TILES_PER_MEGA], F32, tag="sk2",
                               name="sk2_t")
            mx2_t = spool.tile([128, TILES_PER_MEGA], F32, tag="mx2",
                               name="mx2_t")
            sv2_t = spool.tile([128, TILES_PER_MEGA], F32, tag="sv2",
                               name="sv2_t")

            # ===== gather + transpose + k side =====
            for jt_l in range(TILES_PER_MEGA):
                jt = mega * TILES_PER_MEGA + jt_l
                e_j = epool.tile([128, NT * EMBED_DIM], F32, tag="e_j",
                                 name="e_j", bufs=4)
                for i in range(NT):
                    nc.gpsimd.indirect_dma_start(
                        out=e_j[:, i * EMBED_DIM:(i + 1) * EMBED_DIM],
                        out_offset=None,
                        in_=tab_d[:],
                        in_offset=bass.IndirectOffsetOnAxis(
                            ap=idx_col(jt, i), axis=0),
                    )
                if DEBUG_TAPS and jt == 0:
                    nc.sync.dma_start(dbg_e_d[:], e_j[:])
                for q in range(8):
                    tr_ps = trps()
                    nc.tensor.transpose(tr_ps[:],
                                        e_j[:, q * 128:(q + 1) * 128],
                                        ident_f[:])
                    nc.scalar.copy(
                        eT_mega[:, q * MEGA_T + jt_l * 128:
                                q * MEGA_T + jt_l * 128 + 128],
                        tr_ps[:])
                x_j = xpool.tile([128, N_EMBD], F32, tag="x_j", name="x_j",
                                 bufs=2)
                nc.sync.dma_start(x_j[:], x_d[jt * 128:(jt + 1) * 128, :])
                part4 = scr("part4", (128, 4), bufs=2)
                xsq = scr("dvescr", (128, 512), bufs=2)
                for dc in range(4):
                    nc.vector.tensor_tensor_reduce(
                        out=xsq[:], in0=x_j[:, dc * 512:(dc + 1) * 512],
                        in1=x_j[:, dc * 512:(dc + 1) * 512], scale=1.0,
                        scalar=0.0, op0=Alu.mult, op1=Alu.add,
                        accum_out=part4[:, dc:dc + 1])
                nc.vector.tensor_reduce(mx2_t[:, jt_l:jt_l + 1], part4[:],
                                        AxX, Alu.add)
                pk4 = scr("pk4", (128, 4), bufs=2)
                px4 = scr("px4", (128, 4), bufs=2)
                for dc in range(4):
                    k_ps = ps_k.tile([128, 512], F32, space="PSUM", tag="kps",
                                     name="k_ps", bufs=2)
                    for q in range(8):
                        nc.tensor.matmul(
                            k_ps[:],
                            eT_mega[:, q * MEGA_T + jt_l * 128:
                                    q * MEGA_T + jt_l * 128 + 128],
                            wk_sb[q][:, dc * 512:(dc + 1) * 512],
                            start=(q == 0), stop=(q == 7))
                    ksq = scr("ksq", (128, 512), bufs=2)
                    nc.scalar.activation(ksq[:], k_ps[:], Act.Square,
                                         accum_out=pk4[:, dc:dc + 1])
                    xk = scr("dvescr", (128, 512), bufs=2)
                    nc.vector.tensor_tensor_reduce(
                        out=xk[:], in0=k_ps[:],
                        in1=x_j[:, dc * 512:(dc + 1) * 512], scale=1.0,
                        scalar=0.0, op0=Alu.mult, op1=Alu.add,
                        accum_out=px4[:, dc:dc + 1])
                nc.vector.tensor_reduce(sk2_t[:, jt_l:jt_l + 1], pk4[:],
                                        AxX, Alu.add)
                nc.vector.tensor_reduce(sxk_t[:, jt_l:jt_l + 1], px4[:],
                                        AxX, Alu.add)

            # ===== v^T + column sums of v^2 =====
            vT_sb = []
            ssqv_ps = ps_s.tile([8, MEGA_T], F32, space="PSUM", tag="ssqv",
                                name="ssqv_ps", bufs=1)
            for c in range(N_DCH):
                vt_c = vpool.tile([128, MEGA_T], BF16, tag="vtc",
                                  name="vt_c", bufs=16)
                v_ps = ps_v.tile([128, MEGA_T], F32, space="PSUM", tag="vps",
                                 name="v_ps", bufs=1)
                for tcx in range(TC_PER_MEGA):
                    for q in range(8):
                        nc.tensor.matmul(
                            v_ps[:, tcx * 512:(tcx + 1) * 512],
                            wv_sb[q][:, c * 128:(c + 1) * 128],
                            eT_mega[:, q * MEGA_T + tcx * 512:
                                    q * MEGA_T + tcx * 512 + 512],
                            start=(q == 0), stop=(q == 7))
                nc.scalar.copy(vt_c[:], v_ps[:])
                vsq = scr("vsq", (128, MEGA_T), BF16, bufs=2)
                nc.scalar.activation(vsq[:], vt_c[:], Act.Square)
                for tcx in range(TC_PER_MEGA):
                    nc.tensor.matmul(
                        ssqv_ps[0:8, tcx * 512:(tcx + 1) * 512],
                        ones8[:],
                        vsq[:, tcx * 512:(tcx + 1) * 512],
                        start=(c == 0), stop=(c == N_DCH - 1),
                        skip_group_check=True)
                if DEBUG_TAPS and mega == 0 and c == 0:
                    dbg_vtf = scr("dbgvt", (128, MEGA_T), F32)
                    nc.vector.tensor_copy(dbg_vtf[:], vt_c[:])
                    nc.sync.dma_start(dbg_vt_d[:], dbg_vtf[:])
                vT_sb.append(vt_c)
            ssqv8 = spool.tile([8, MEGA_T], F32, tag="sq8", name="ssqv8")
            nc.scalar.copy(ssqv8[:], ssqv_ps[:])
            # transpose diagonal blocks -> sv2_t[:, jt_l]
            for jt_l in range(TILES_PER_MEGA):
                sv_ps = trps((128, 8))
                nc.tensor.matmul(sv_ps[:],
                                 ssqv8[:, jt_l * 128:(jt_l + 1) * 128],
                                 ident_f[0:8, 0:8],
                                 is_transpose=True, start=True, stop=True)
                nc.vector.tensor_copy(sv2_t[:, jt_l:jt_l + 1], sv_ps[:, 0:1])

            # ===== per-token scalars [128, 8] =====
            def sp(nm):
                return spool.tile([128, TILES_PER_MEGA], F32, tag=nm, name=nm)

            m1 = sp("m1")
            nc.vector.tensor_scalar(m1[:], mx2_t[:], 1.0 / N_EMBD, EPS,
                                    Alu.mult, Alu.add)
            m2s = sp("m2s")
            nc.vector.tensor_scalar(m2s[:], sk2_t[:], EPS * N_EMBD, None,
                                    Alu.add)
            pr = sp("pr")
            nc.vector.tensor_tensor(pr[:], m1[:], m2s[:], Alu.mult)
            rp = sp("rp")
            nc.vector.reciprocal(rp[:], pr[:])
            rt = sp("rt")
            nc.scalar.sqrt(rt[:], rp[:])
            zg = sp("zg")
            nc.vector.tensor_tensor(zg[:], sxk_t[:], rt[:], Alu.mult)
            gate = sp("gate")
            nc.scalar.activation(gate[:], zg[:], Act.Sigmoid)
            g2v = sp("g2v")
            nc.vector.tensor_tensor(g2v[:], gate[:], gate[:], Alu.mult)
            nc.vector.tensor_tensor(g2v[:], g2v[:], sv2_t[:], Alu.mult)
            m3 = sp("m3")
            nc.vector.tensor_scalar(m3[:], g2v[:], 1.0 / N_EMBD, EPS,
                                    Alu.mult, Alu.add)
            irg = sp("irg")
            nc.scalar.sqrt(irg[:], m3[:])       # = 1/r_gv
            rq = sp("rq")
            nc.vector.reciprocal(rq[:], m3[:])
            rgv = sp("rgv")
            nc.scalar.sqrt(rgv[:], rq[:])       # = r_gv
            snv = sp("snv")
            nc.vector.tensor_tensor(snv[:], gate[:], rgv[:], Alu.mult)
            if DEBUG_TAPS and mega == 0:
                for di, tl in enumerate([sxk_t, sk2_t, mx2_t, sv2_t, gate,
                                         rgv, snv, irg]):
                    nc.sync.dma_start(dbg_sc_d[di], tl[:])

            # ===== replicate snv / irg along d (bf16 [128, MEGA_T]) =====
            def replicate(sc_tile, nm):
                pad_in = scr("repin", (128, 128))
                nc.vector.memset(pad_in[:], 0.0)
                nc.vector.tensor_copy(pad_in[:, 0:TILES_PER_MEGA], sc_tile[:])
                tp = trps()
                nc.tensor.transpose(tp[:], pad_in[:], ident_f[:])
                row = scr("reprow", (128, 128), BF16)
                nc.scalar.copy(row[:], tp[:])
                # compact partitions 0..7 into one partition-0 row
                row8 = scr("reprow8", (1, TILES_PER_MEGA * 128), BF16)
                nc.sync.dma_start(
                    row8[:].rearrange("o (j t) -> o j t",
                                      j=TILES_PER_MEGA),
                    row[0:TILES_PER_MEGA, :])
                rep = dwork.tile([128, MEGA_T], BF16, tag=nm, name=nm, bufs=1)
                for jt_l in range(TILES_PER_MEGA):
                    rp_ps = trps()
                    nc.tensor.matmul(rp_ps[:], ones_row[:],
                                     row8[0:1, jt_l * 128:(jt_l + 1) * 128],
                                     start=True, stop=True)
                    nc.scalar.copy(rep[:, jt_l * 128:(jt_l + 1) * 128],
                                   rp_ps[:])
                return rep

            snv_rep = replicate(snv, "snv_rep")
            irg_rep = replicate(irg, "irg_rep")
            if DEBUG_TAPS and mega == 0:
                for di, rp_ in enumerate([snv_rep, irg_rep]):
                    dbg_rf = scr("dbgrf", (128, MEGA_T), F32, bufs=1)
                    nc.vector.tensor_copy(dbg_rf[:], rp_[:])
                    nc.sync.dma_start(dbg_rep_d[di], dbg_rf[:])

            # ===== conv + output =====
            for c in range(N_DCH):
                vt_c = vT_sb[c]
                nvx = dwork.tile([128, PAD + MEGA_T], BF16, tag="nvx",
                                 name="nvx", bufs=2)
                nc.vector.tensor_copy(nvx[:, 0:PAD], tails[c][:])
                nc.vector.tensor_tensor(nvx[:, PAD:], vt_c[:], snv_rep[:],
                                        Alu.mult)
                nc.vector.tensor_copy(tails[c][:], nvx[:, MEGA_T:])
                cacc = dwork.tile([128, MEGA_T], BF16, tag="cacc",
                                  name="cacc", bufs=2)
                nc.vector.tensor_scalar(cacc[:], nvx[:, 0:MEGA_T],
                                        cw[:, c * 4:c * 4 + 1], None,
                                        Alu.mult)
                for i in range(1, KERNEL):
                    nc.vector.scalar_tensor_tensor(
                        cacc[:], nvx[:, 3 * i:3 * i + MEGA_T],
                        cw[:, c * 4 + i:c * 4 + i + 1], cacc[:],
                        Alu.mult, Alu.add)
                if DEBUG_TAPS and mega == 0 and c == 0:
                    dbg_nf = scr("dbgnf", (128, PAD + MEGA_T), F32, bufs=1)
                    nc.vector.tensor_copy(dbg_nf[:], nvx[:])
                    nc.sync.dma_start(dbg_nv_d[:], dbg_nf[:])
                    dbg_cf = scr("dbgcf", (128, MEGA_T), F32, bufs=1)
                    nc.vector.tensor_copy(dbg_cf[:], cacc[:])
                    nc.sync.dma_start(dbg_cv_d[:], dbg_cf[:])
                nc.scalar.activation(cacc[:], cacc[:], Act.Silu)
                # gv overwrites vt_c (dead after the nvx mult), then += silu
                nc.vector.tensor_tensor(vt_c[:], nvx[:, PAD:], irg_rep[:],
                                        Alu.mult)
                nc.vector.tensor_tensor(vt_c[:], vt_c[:], cacc[:], Alu.add)

            if DEBUG_TAPS and mega == 0:
                dbg_sf = scr("dbgsf", (128, MEGA_T), F32, bufs=1)
                nc.vector.tensor_copy(dbg_sf[:], vT_sb[0][:])
                nc.sync.dma_start(dbg_sg_d[:], dbg_sf[:])
            # ===== transpose to [t, d] rows and write out =====
            for tg in range(TILES_PER_MEGA):
                for half in range(2):
                    o_row = obuf.tile([128, N_EMBD // 2], F32, tag="orow",
                                      name="o_row", bufs=2)
                    for b4 in range(2):
                        o_ps = ps_o.tile([128, 512], BF16, space="PSUM",
                                         tag="ops", name="o_ps", bufs=2)
                        for c4 in range(4):
                            c = half * 8 + b4 * 4 + c4
                            nc.tensor.matmul(
                                o_ps[:, c4 * 128:(c4 + 1) * 128],
                                vT_sb[c][:, tg * 128:(tg + 1) * 128],
                                ident_b[:], is_transpose=True,
                                start=True, stop=True,
                                skip_group_check=True)
                        nc.scalar.copy(o_row[:, b4 * 512:(b4 + 1) * 512],
                                       o_ps[:])
                    trow = mega * MEGA_T + tg * 128
                    nc.sync.dma_start(
                        out_d[trow:trow + 128,
                              half * 1024:(half + 1) * 1024],
                        o_row[:])

    nc.compile()
    return nc


_CACHED = None


def _get_program():
    global _CACHED
    if _CACHED is None:
        _CACHED = build_program()
    return _CACHED


def make_in_maps(x, input_ids, tables, key_w, value_w, conv_w):
    x = np.asarray(x, np.float32)
    input_ids = np.asarray(input_ids, np.int32)
    tables = np.asarray(tables, np.float32)
    key_w = np.asarray(key_w, np.float32)
    value_w = np.asarray(value_w, np.float32)
    conv_w = np.asarray(conv_w, np.float32)

    wkT = np.ascontiguousarray(key_w.T).astype(ml_dtypes.bfloat16)
    wvT = np.ascontiguousarray(value_w.T).astype(ml_dtypes.bfloat16)
    conv_pp = np.zeros((128, N_DCH * KERNEL), np.float32)
    for c in range(N_DCH):
        for i in range(KERNEL):
            conv_pp[:, c * KERNEL + i] = conv_w[c * 128:(c + 1) * 128, 0, i]

    in_maps = []
    for b in range(B):
        # s_j[(i*8+g), t'] = ids[g*256 + t' - (2 - j)], zero for negative
        ids_pad = np.concatenate([np.zeros(2, np.int32), input_ids[b]])
        sj_all = np.zeros((3, 128, 256), np.int32)
        for j in range(3):
            shifted = ids_pad[j:j + T].reshape(8, 256)  # [g, t']
            sj_all[j] = np.tile(shifted, (16, 1))
        in_maps.append({
            "sj_all": sj_all,
            "tables": tables,
            "xrow": np.ascontiguousarray(x[b]),
            "wkT": wkT,
            "wvT": wvT,
            "consts_pp": CONSTS_PP,
            "conv_pp": conv_pp,
        })
    return in_maps


def host_reference_hash(input_ids):
    """Exact gather indices, [B, NT, T] int64 (for testing)."""
    ids = np.asarray(input_ids, np.int64)
    Bx, Tx = ids.shape
    out = np.zeros((Bx, NT, Tx), np.int64)
    ids_pad = np.concatenate([np.zeros((Bx, 2), np.int64), ids], axis=1)
    for i in range(NT):
        p = PRIMES[i]
        acc = np.zeros((Bx, Tx), np.int64)
        for j in range(3):
            s = ids_pad[:, j:j + Tx]
            acc = (acc + (s * W_UNI[i][j]) % p) % p
        out[:, i, :] = acc + int(OFFSETS[i])
    return out


def kernel(x, input_ids, tables, key_w, value_w, conv_w):
    nc = _get_program()
    in_maps = make_in_maps(x, input_ids, tables, key_w, value_w, conv_w)
    res = bass_utils.run_bass_kernel_spmd(nc, in_maps, core_ids=list(range(B)))
    out = np.stack([res.results[b]["out"] for b in range(B)], axis=0)
    return out.astype(np.float32)


# revision 31
# speedup vs baseline: 1.0730x; 1.0730x over previous
"""Trainium2 Bass kernel for nn_EngramModule (hash-ngram embedding gating).

Sharding: data-parallel over batch across 8 NeuronCores (one batch row per
core); hash tables and weights replicated per core.

Per-core pipeline:
  - exact n-gram hashes on DVE in fp32 (fused multiply / round-to-nearest /
    two-term subtract modular reduction; no integer mod on the DVE ISA)
  - hash tensor PE-transposed so gather indices are plain column slices
  - 256 indirect DMAs gather embedding rows (128 rows of 256B each)
  - e PE-transposed to m-major; k = e @ key_w^T and v^T = value_w^T @ e^T
    run as bf16 matmuls (full PE rate), fp32 PSUM accumulate
  - per-token scalars: free-dim reductions (k side), PE ones-matmul (v side)
  - dilated causal depthwise conv: 4 shifted MACs along the free token axis
    in [d, t] layout with per-partition channel weights
  - out = silu(conv) + gate*v assembled by accumulating two PE transposes
    per block in PSUM, escaping fp32 to [t, d]
"""
from contextlib import ExitStack

import numpy as np
import ml_dtypes

import concourse.bass as bass
import concourse.mybir as mybir
import concourse.tile as tile
from concourse import bacc, bass_utils
from concourse.masks import make_identity

F32 = mybir.dt.float32
BF16 = mybir.dt.bfloat16
I32 = mybir.dt.int32
Alu = mybir.AluOpType
Act = mybir.ActivationFunctionType
AxX = mybir.AxisListType.X

# ---- module config (mirrors the reference) ----
MAX_NGRAM = 3
N_HEADS = 8
EMBED_DIM = 64
N_EMBD = 2048
KERNEL = 4
DILATION = MAX_NGRAM
VOCAB = 32768
BASE_TABLE = 3 * VOCAB
B, T = 8, 2048
D_MEM = (MAX_NGRAM - 1) * N_HEADS * EMBED_DIM  # 1024
EPS = float(np.finfo(np.float32).eps)


def _next_prime(n):
    if n <= 2:
        return 2
    if n % 2 == 0:
        n += 1
    while True:
        if all(n % d for d in range(3, int(n ** 0.5) + 1, 2)):
            return n
        n += 2


PRIMES, POWERS, NS = [], [], []
for _n in range(2, MAX_NGRAM + 1):
    for _k in range(N_HEADS):
        _p = _next_prime(BASE_TABLE + _n * _k * 997)
        _seed = _n * 2654435761 + _k * 40503
        PRIMES.append(_p)
        POWERS.append([pow(_seed, _n - _j, _p) for _j in range(_n)])
        NS.append(_n)
OFFSETS = np.concatenate([[0], np.cumsum(PRIMES)]).astype(np.int64)
TOTAL_ROWS = int(OFFSETS[-1])
NT = 16

# unified 3-tap hash weights; tap j multiplies ids[tau - (2 - j)]
W_UNI = []
for _i in range(NT):
    if NS[_i] == 2:
        W_UNI.append([0, POWERS[_i][0], POWERS[_i][1]])
    else:
        W_UNI.append(list(POWERS[_i]))

MAGIC = float(3 * 2 ** 22)
N_MEGA = 4
MEGA_T = T // N_MEGA
TILES_PER_MEGA = MEGA_T // 128   # 8
N_TILES = T // 128               # 16
N_DCH = N_EMBD // 128            # 16
TC_PER_MEGA = MEGA_T // 512      # 2
PAD = DILATION * (KERNEL - 1)    # 9


def _host_constants():
    part_i = np.arange(128) // 8

    def col(vals):
        return np.asarray(vals, np.float64)[part_i].astype(np.float32)

    order = []
    cols = []

    def add(name, vals):
        order.append(name)
        cols.append(col(vals))

    for j in range(3):
        add(f"a{j}", [W_UNI[i][j] >> 8 for i in range(NT)])
    for j in range(3):
        add(f"b{j}", [W_UNI[i][j] & 255 for i in range(NT)])
    add("invp", [np.float32(1.0) / np.float32(PRIMES[i]) for i in range(NT)])
    add("ph256", [(PRIMES[i] >> 8) * 256 for i in range(NT)])
    add("pl", [PRIMES[i] & 255 for i in range(NT)])
    add("p", [PRIMES[i] for i in range(NT)])
    add("negp", [-PRIMES[i] for i in range(NT)])
    add("off", [int(OFFSETS[i]) for i in range(NT)])
    arr = np.stack(cols, axis=1)
    return arr, {k: j for j, k in enumerate(order)}


CONSTS_PP, CIDX = _host_constants()


DEBUG_TAPS = False


def build_program():
    nc = bacc.Bacc("TRN2", target_bir_lowering=False, debug=False)

    sj_d = nc.dram_tensor("sj_all", [3, 128, 256], I32,
                          kind="ExternalInput").ap()
    tab_d = nc.dram_tensor("tables", [TOTAL_ROWS, EMBED_DIM], F32,
                           kind="ExternalInput").ap()
    x_d = nc.dram_tensor("xrow", [T, N_EMBD], F32, kind="ExternalInput").ap()
    wk_d = nc.dram_tensor("wkT", [D_MEM, N_EMBD], BF16,
                          kind="ExternalInput").ap()
    wv_d = nc.dram_tensor("wvT", [D_MEM, N_EMBD], BF16,
                          kind="ExternalInput").ap()
    cpp_d = nc.dram_tensor("consts_pp", [128, CONSTS_PP.shape[1]], F32,
                           kind="ExternalInput").ap()
    cw_d = nc.dram_tensor("conv_pp", [128, N_DCH * KERNEL], F32,
                          kind="ExternalInput").ap()
    out_d = nc.dram_tensor("out", [T, N_EMBD], F32, kind="ExternalOutput").ap()
    if DEBUG_TAPS:
        dbg_idx_d = nc.dram_tensor("dbg_idx", [2, 128, 128], I32,
                                   kind="ExternalOutput").ap()
        dbg_e_d = nc.dram_tensor("dbg_e", [128, NT * EMBED_DIM], F32,
                                 kind="ExternalOutput").ap()
        dbg_sc_d = nc.dram_tensor("dbg_sc", [8, 128, TILES_PER_MEGA], F32,
                                  kind="ExternalOutput").ap()
        dbg_vt_d = nc.dram_tensor("dbg_vt", [128, MEGA_T], F32,
                                  kind="ExternalOutput").ap()
        dbg_nv_d = nc.dram_tensor("dbg_nv", [128, PAD + MEGA_T], F32,
                                  kind="ExternalOutput").ap()
        dbg_cv_d = nc.dram_tensor("dbg_cv", [128, MEGA_T], F32,
                                  kind="ExternalOutput").ap()
        dbg_sg_d = nc.dram_tensor("dbg_sg", [128, MEGA_T], F32,
                                  kind="ExternalOutput").ap()
        dbg_rep_d = nc.dram_tensor("dbg_rep", [2, 128, MEGA_T], F32,
                                   kind="ExternalOutput").ap()

    with tile.TileContext(nc) as tc, ExitStack() as ctx:
        cpool = ctx.enter_context(tc.tile_pool(name="cpool", bufs=1))
        ps_tr = ctx.enter_context(tc.tile_pool(name="ps_tr", bufs=2,
                                               space="PSUM"))
        ps_k = ctx.enter_context(tc.tile_pool(name="ps_k", bufs=2,
                                              space="PSUM"))
        ps_v = ctx.enter_context(tc.tile_pool(name="ps_v", bufs=2,
                                              space="PSUM"))
        ps_s = ctx.enter_context(tc.tile_pool(name="ps_s", bufs=1,
                                              space="PSUM"))
        ps_o = ctx.enter_context(tc.tile_pool(name="ps_o", bufs=1,
                                              space="PSUM"))

        scrp = None

        def scr(tag, shape=(128, 256), dtype=F32, bufs=1, pool=None):
            return (pool or scrp).tile(list(shape), dtype, tag=tag, name=tag,
                                       bufs=bufs)

        def trps(shape=(128, 128)):
            return ps_tr.tile(list(shape), F32, space="PSUM", tag="trp",
                              name="trp", bufs=2)

        # ---------- constants ----------
        wk_sb, wv_sb = [], []
        for q in range(8):
            wkq = cpool.tile([128, N_EMBD], BF16, name=f"wk{q}")
            nc.sync.dma_start(wkq[:], wk_d[q * 128:(q + 1) * 128, :])
            wk_sb.append(wkq)
            wvq = cpool.tile([128, N_EMBD], BF16, name=f"wv{q}")
            nc.sync.dma_start(wvq[:], wv_d[q * 128:(q + 1) * 128, :])
            wv_sb.append(wvq)
        cpp = cpool.tile([128, CONSTS_PP.shape[1]], F32, name="cpp")
        nc.sync.dma_start(cpp[:], cpp_d[:])
        cw = cpool.tile([128, N_DCH * KERNEL], F32, name="cw")
        nc.sync.dma_start(cw[:], cw_d[:])
        ident_f = cpool.tile([128, 128], F32, name="ident_f")
        make_identity(nc, ident_f[:])
        ident_b = cpool.tile([128, 128], BF16, name="ident_b")
        nc.vector.tensor_copy(ident_b[:], ident_f[:])
        ones_col = cpool.tile([128, 1], BF16, name="ones_col")
        nc.vector.memset(ones_col[:], 1.0)
        ones_row = cpool.tile([1, 128], BF16, name="ones_row")
        nc.vector.memset(ones_row[:], 1.0)
        ones8 = cpool.tile([128, 8], BF16, name="ones8")
        nc.vector.memset(ones8[:], 1.0)

        def cc(name):
            return cpp[:, CIDX[name]:CIDX[name] + 1]

        # ---------- hash ----------
        hctx = ExitStack()
        hpool = hctx.enter_context(tc.tile_pool(name="hpool", bufs=1))
        s_f = []
        for j in range(3):
            sj_i = hpool.tile([128, 256], I32, name=f"s{j}_i")
            nc.sync.dma_start(sj_i[:], sj_d[j])
            sj_f = hpool.tile([128, 256], F32, name=f"s{j}_f")
            nc.vector.tensor_copy(sj_f[:], sj_i[:])
            s_f.append(sj_f)

        def reduce4(x):
            q = scr("hq", pool=hpool)
            nc.vector.tensor_scalar(q[:], x[:], cc("invp"), None, Alu.mult)
            nc.vector.tensor_scalar(q[:], q[:], MAGIC, MAGIC, Alu.add,
                                    Alu.subtract)
            r = scr("hr" + x.name[-2:], bufs=1, pool=hpool)
            nc.vector.scalar_tensor_tensor(r[:], q[:], cc("ph256"), x[:],
                                           Alu.mult, Alu.subtract)
            nc.vector.scalar_tensor_tensor(r[:], q[:], cc("pl"), r[:],
                                           Alu.mult, Alu.add)
            return r

        m1s, los = [], []
        for j in range(3):
            t_j = scr(f"ht{j}", pool=hpool)
            nc.vector.tensor_scalar(t_j[:], s_f[j][:], cc(f"a{j}"), None,
                                    Alu.mult)
            m1s.append(reduce4(t_j))
            lo = scr(f"hlo{j}", pool=hpool)
            nc.vector.tensor_scalar(lo[:], s_f[j][:], cc(f"b{j}"), None,
                                    Alu.mult)
            los.append(lo)
        U = scr("hU", pool=hpool)
        nc.vector.tensor_tensor(U[:], m1s[0][:], m1s[1][:], Alu.add)
        nc.vector.tensor_tensor(U[:], U[:], m1s[2][:], Alu.add)
        gsc = scr("hgs", pool=hpool)
        nc.vector.tensor_scalar(gsc[:], U[:], -256.0, None, Alu.mult)
        m2 = reduce4(gsc)
        L2 = scr("hL2", pool=hpool)
        nc.vector.tensor_tensor(L2[:], los[0][:], los[1][:], Alu.add)
        Lr = reduce4(L2)
        Hh = scr("hHa", pool=hpool)
        nc.vector.tensor_tensor(Hh[:], m2[:], Lr[:], Alu.add)
        nc.vector.tensor_tensor(Hh[:], Hh[:], los[2][:], Alu.subtract)
        Fh = reduce4(Hh)
        msk = scr("hmk", pool=hpool)
        for _ in range(2):
            nc.vector.tensor_scalar(msk[:], Fh[:], 0.0, None, Alu.is_lt)
            nc.vector.scalar_tensor_tensor(msk[:], msk[:], cc("p"), Fh[:],
                                           Alu.mult, Alu.add)
            Fh, msk = msk, Fh
        nc.vector.tensor_scalar(msk[:], Fh[:], cc("p"), None, Alu.is_ge)
        nc.vector.scalar_tensor_tensor(msk[:], msk[:], cc("negp"), Fh[:],
                                       Alu.mult, Alu.add)
        Fh, msk = msk, Fh
        nc.vector.tensor_scalar(Fh[:], Fh[:], cc("off"), None, Alu.add)

        hT_i32 = []
        for h in range(2):
            hT_ps = trps()
            nc.tensor.transpose(hT_ps[:], Fh[:, h * 128:(h + 1) * 128],
                                ident_f[:])
            hTi = cpool.tile([128, 128], I32, name=f"hTi{h}")
            nc.vector.tensor_copy(hTi[:], hT_ps[:])
            hT_i32.append(hTi)

        if DEBUG_TAPS:
            for h in range(2):
                nc.sync.dma_start(dbg_idx_d[h], hT_i32[h][:])
        hctx.close()

        scrp = ctx.enter_context(tc.tile_pool(name="scrp", bufs=1))
        epool = ctx.enter_context(tc.tile_pool(name="epool", bufs=4))
        etm_p = ctx.enter_context(tc.tile_pool(name="etm_p", bufs=2))
        xpool = ctx.enter_context(tc.tile_pool(name="xpool", bufs=3))
        vpool = ctx.enter_context(tc.tile_pool(name="vpool", bufs=16))
        dwork = ctx.enter_context(tc.tile_pool(name="dwork", bufs=2))
        spool = ctx.enter_context(tc.tile_pool(name="spool", bufs=2))
        obuf = ctx.enter_context(tc.tile_pool(name="obuf", bufs=4))
        tailp = ctx.enter_context(tc.tile_pool(name="tailp", bufs=1))

        def idx_col(jt, i):
            h, g = jt % 2, jt // 2
            return hT_i32[h][:, i * 8 + g: i * 8 + g + 1]

        # ---------- per-d-chunk conv tails ----------
        tails = []
        for c in range(N_DCH):
            tl = tailp.tile([128, PAD], BF16, name=f"tail{c}")
            nc.vector.memset(tl[:], 0.0)
            tails.append(tl)

        for mega in range(N_MEGA):
            eT_mega = etm_p.tile([128, 8 * MEGA_T], BF16, tag="eTm",
                                 name="eT_mega", bufs=2)
            sxk_t = spool.tile([128, TILES_PER_MEGA], F32, tag="sxk",
                               name="sxk_t")
            sk2_t = spool.tile([128, TILES_PER_MEGA], F32, tag="sk2",
                               name="sk2_t")
            mx2_t = spool.tile([128, TILES_PER_MEGA], F32, tag="mx2",
                               name="mx2_t")
            sv2_t = spool.tile([128, TILES_PER_MEGA], F32, tag="sv2",
                               name="sv2_t")

            # ===== gather + transpose + k side =====
            for jt_l in range(TILES_PER_MEGA):
                jt = mega * TILES_PER_MEGA + jt_l
                e_j = epool.tile([128, NT * EMBED_DIM], F32, tag="e_j",
                                 name="e_j", bufs=4)
                for i in range(NT):
                    nc.gpsimd.indirect_dma_start(
                        out=e_j[:, i * EMBED_DIM:(i + 1) * EMBED_DIM],
                        out_offset=None,
                        in_=tab_d[:],
                        in_offset=bass.IndirectOffsetOnAxis(
                            ap=idx_col(jt, i), axis=0),
                    )
                if DEBUG_TAPS and jt == 0:
                    nc.sync.dma_start(dbg_e_d[:], e_j[:])
                for q in range(8):
                    tr_ps = trps()
                    nc.tensor.transpose(tr_ps[:],
                                        e_j[:, q * 128:(q + 1) * 128],
                                        ident_f[:])
                    nc.scalar.copy(
                        eT_mega[:, q * MEGA_T + jt_l * 128:
                                q * MEGA_T + jt_l * 128 + 128],
                        tr_ps[:])
                x_j = xpool.tile([128, N_EMBD], F32, tag="x_j", name="x_j",
                                 bufs=2)
                nc.sync.dma_start(x_j[:], x_d[jt * 128:(jt + 1) * 128, :])
                part4 = scr("part4", (128, 4), bufs=2)
                xsq = scr("dvescr", (128, 512), bufs=2)
                for dc in range(4):
                    nc.vector.tensor_tensor_reduce(
                        out=xsq[:], in0=x_j[:, dc * 512:(dc + 1) * 512],
                        in1=x_j[:, dc * 512:(dc + 1) * 512], scale=1.0,
                        scalar=0.0, op0=Alu.mult, op1=Alu.add,
                        accum_out=part4[:, dc:dc + 1])
                nc.vector.tensor_reduce(mx2_t[:, jt_l:jt_l + 1], part4[:],
                                        AxX, Alu.add)
                pk4 = scr("pk4", (128, 4), bufs=2)
                px4 = scr("px4", (128, 4), bufs=2)
                for dc in range(4):
                    k_ps = ps_k.tile([128, 512], F32, space="PSUM", tag="kps",
                                     name="k_ps", bufs=2)
                    for q in range(8):
                        nc.tensor.matmul(
                            k_ps[:],
                            eT_mega[:, q * MEGA_T + jt_l * 128:
                                    q * MEGA_T + jt_l * 128 + 128],
                            wk_sb[q][:, dc * 512:(dc + 1) * 512],
                            start=(q == 0), stop=(q == 7))
                    ksq = scr("ksq", (128, 512), bufs=2)
                    nc.scalar.activation(ksq[:], k_ps[:], Act.Square,
                                         accum_out=pk4[:, dc:dc + 1])
                    xk = scr("dvescr", (128, 512), bufs=2)
                    nc.vector.tensor_tensor_reduce(
                        out=xk[:], in0=k_ps[:],
                        in1=x_j[:, dc * 512:(dc + 1) * 512], scale=1.0,
                        scalar=0.0, op0=Alu.mult, op1=Alu.add,
                        accum_out=px4[:, dc:dc + 1])
                nc.vector.tensor_reduce(sk2_t[:, jt_l:jt_l + 1], pk4[:],
                                        AxX, Alu.add)
                nc.vector.tensor_reduce(sxk_t[:, jt_l:jt_l + 1], px4[:],
                                        AxX, Alu.add)

            # ===== v^T + column sums of v^2 =====
            vT_sb = []
            ssqv_ps = ps_s.tile([8, MEGA_T], F32, space="PSUM", tag="ssqv",
                                name="ssqv_ps", bufs=1)
            for c in range(N_DCH):
                vt_c = vpool.tile([128, MEGA_T], BF16, tag="vtc",
                                  name="vt_c", bufs=16)
                v_ps = ps_v.tile([128, MEGA_T], F32, space="PSUM", tag="vps",
                                 name="v_ps", bufs=2)
                for tcx in range(TC_PER_MEGA):
                    for q in range(8):
                        nc.tensor.matmul(
                            v_ps[:, tcx * 512:(tcx + 1) * 512],
                            wv_sb[q][:, c * 128:(c + 1) * 128],
                            eT_mega[:, q * MEGA_T + tcx * 512:
                                    q * MEGA_T + tcx * 512 + 512],
                            start=(q == 0), stop=(q == 7))
                nc.scalar.copy(vt_c[:], v_ps[:])
                vsq = scr("vsq", (128, MEGA_T), BF16, bufs=2)
                nc.scalar.activation(vsq[:], vt_c[:], Act.Square)
                for tcx in range(TC_PER_MEGA):
                    nc.tensor.matmul(
                        ssqv_ps[0:8, tcx * 512:(tcx + 1) * 512],
                        ones8[:],
                        vsq[:, tcx * 512:(tcx + 1) * 512],
                        start=(c == 0), stop=(c == N_DCH - 1),
                        skip_group_check=True)
                if DEBUG_TAPS and mega == 0 and c == 0:
                    dbg_vtf = scr("dbgvt", (128, MEGA_T), F32)
                    nc.vector.tensor_copy(dbg_vtf[:], vt_c[:])
                    nc.sync.dma_start(dbg_vt_d[:], dbg_vtf[:])
                vT_sb.append(vt_c)
            ssqv8 = spool.tile([8, MEGA_T], F32, tag="sq8", name="ssqv8")
            nc.scalar.copy(ssqv8[:], ssqv_ps[:])
            # transpose diagonal blocks -> sv2_t[:, jt_l]
            for jt_l in range(TILES_PER_MEGA):
                sv_ps = trps((128, 8))
                nc.tensor.matmul(sv_ps[:],
                                 ssqv8[:, jt_l * 128:(jt_l + 1) * 128],
                                 ident_f[0:8, 0:8],
                                 is_transpose=True, start=True, stop=True)
                nc.vector.tensor_copy(sv2_t[:, jt_l:jt_l + 1], sv_ps[:, 0:1])

            # ===== per-token scalars [128, 8] =====
            def sp(nm):
                return spool.tile([128, TILES_PER_MEGA], F32, tag=nm, name=nm)

            m1 = sp("m1")
            nc.vector.tensor_scalar(m1[:], mx2_t[:], 1.0 / N_EMBD, EPS,
                                    Alu.mult, Alu.add)
            m2s = sp("m2s")
            nc.vector.tensor_scalar(m2s[:], sk2_t[:], EPS * N_EMBD, None,
                                    Alu.add)
            pr = sp("pr")
            nc.vector.tensor_tensor(pr[:], m1[:], m2s[:], Alu.mult)
            rp = sp("rp")
            nc.vector.reciprocal(rp[:], pr[:])
            rt = sp("rt")
            nc.scalar.sqrt(rt[:], rp[:])
            zg = sp("zg")
            nc.vector.tensor_tensor(zg[:], sxk_t[:], rt[:], Alu.mult)
            gate = sp("gate")
            nc.scalar.activation(gate[:], zg[:], Act.Sigmoid)
            g2v = sp("g2v")
            nc.vector.tensor_tensor(g2v[:], gate[:], gate[:], Alu.mult)
            nc.vector.tensor_tensor(g2v[:], g2v[:], sv2_t[:], Alu.mult)
            m3 = sp("m3")
            nc.vector.tensor_scalar(m3[:], g2v[:], 1.0 / N_EMBD, EPS,
                                    Alu.mult, Alu.add)
            irg = sp("irg")
            nc.scalar.sqrt(irg[:], m3[:])       # = 1/r_gv
            rq = sp("rq")
            nc.vector.reciprocal(rq[:], m3[:])
            rgv = sp("rgv")
            nc.scalar.sqrt(rgv[:], rq[:])       # = r_gv
            snv = sp("snv")
            nc.vector.tensor_tensor(snv[:], gate[:], rgv[:], Alu.mult)
            if DEBUG_TAPS and mega == 0:
                for di, tl in enumerate([sxk_t, sk2_t, mx2_t, sv2_t, gate,
                                         rgv, snv, irg]):
                    nc.sync.dma_start(dbg_sc_d[di], tl[:])

            # ===== replicate snv / irg along d (bf16 [128, MEGA_T]) =====
            def replicate(sc_tile, nm):
                pad_in = scr("repin", (128, 128))
                nc.vector.memset(pad_in[:], 0.0)
                nc.vector.tensor_copy(pad_in[:, 0:TILES_PER_MEGA], sc_tile[:])
                tp = trps()
                nc.tensor.transpose(tp[:], pad_in[:], ident_f[:])
                row = scr("reprow", (128, 128), BF16)
                nc.scalar.copy(row[:], tp[:])
                # compact partitions 0..7 into one partition-0 row
                row8 = scr("reprow8", (1, TILES_PER_MEGA * 128), BF16)
                nc.sync.dma_start(
                    row8[:].rearrange("o (j t) -> o j t",
                                      j=TILES_PER_MEGA),
                    row[0:TILES_PER_MEGA, :])
                rep = dwork.tile([128, MEGA_T], BF16, tag=nm, name=nm, bufs=1)
                for jt_l in range(TILES_PER_MEGA):
                    rp_ps = trps()
                    nc.tensor.matmul(rp_ps[:], ones_row[:],
                                     row8[0:1, jt_l * 128:(jt_l + 1) * 128],
                                     start=True, stop=True)
                    nc.scalar.copy(rep[:, jt_l * 128:(jt_l + 1) * 128],
                                   rp_ps[:])
                return rep

            snv_rep = replicate(snv, "snv_rep")
            irg_rep = replicate(irg, "irg_rep")
            if DEBUG_TAPS and mega == 0:
                for di, rp_ in enumerate([snv_rep, irg_rep]):
                    dbg_rf = scr("dbgrf", (128, MEGA_T), F32, bufs=1)
                    nc.vector.tensor_copy(dbg_rf[:], rp_[:])
                    nc.sync.dma_start(dbg_rep_d[di], dbg_rf[:])

            # ===== conv + output =====
            for c in range(N_DCH):
                vt_c = vT_sb[c]
                nvx = dwork.tile([128, PAD + MEGA_T], BF16, tag="nvx",
                                 name="nvx", bufs=2)
                nc.vector.tensor_copy(nvx[:, 0:PAD], tails[c][:])
                nc.vector.tensor_tensor(nvx[:, PAD:], vt_c[:], snv_rep[:],
                                        Alu.mult)
                nc.vector.tensor_copy(tails[c][:], nvx[:, MEGA_T:])
                cacc = dwork.tile([128, MEGA_T], BF16, tag="cacc",
                                  name="cacc", bufs=2)
                nc.vector.tensor_scalar(cacc[:], nvx[:, 0:MEGA_T],
                                        cw[:, c * 4:c * 4 + 1], None,
                                        Alu.mult)
                for i in range(1, KERNEL):
                    nc.vector.scalar_tensor_tensor(
                        cacc[:], nvx[:, 3 * i:3 * i + MEGA_T],
                        cw[:, c * 4 + i:c * 4 + i + 1], cacc[:],
                        Alu.mult, Alu.add)
                if DEBUG_TAPS and mega == 0 and c == 0:
                    dbg_nf = scr("dbgnf", (128, PAD + MEGA_T), F32, bufs=1)
                    nc.vector.tensor_copy(dbg_nf[:], nvx[:])
                    nc.sync.dma_start(dbg_nv_d[:], dbg_nf[:])
                    dbg_cf = scr("dbgcf", (128, MEGA_T), F32, bufs=1)
                    nc.vector.tensor_copy(dbg_cf[:], cacc[:])
                    nc.sync.dma_start(dbg_cv_d[:], dbg_cf[:])
                nc.scalar.activation(cacc[:], cacc[:], Act.Silu)
                # gv overwrites vt_c (dead after the nvx mult), then += silu
                nc.vector.tensor_tensor(vt_c[:], nvx[:, PAD:], irg_rep[:],
                                        Alu.mult)
                nc.vector.tensor_tensor(vt_c[:], vt_c[:], cacc[:], Alu.add)

            if DEBUG_TAPS and mega == 0:
                dbg_sf = scr("dbgsf", (128, MEGA_T), F32, bufs=1)
                nc.vector.tensor_copy(dbg_sf[:], vT_sb[0][:])
                nc.sync.dma_start(dbg_sg_d[:], dbg_sf[:])
            # ===== transpose to [t, d] rows and write out =====
            for tg in range(TILES_PER_MEGA):
                for half in range(2):
                    o_row = obuf.tile([128, N_EMBD // 2], F32, tag="orow",
                                      name="o_row", bufs=2)
                    for b4 in range(2):
                        o_ps = ps_o.tile([128, 512], BF16, space="PSUM",
                                         tag="ops", name="o_ps", bufs=1)
                        for c4 in range(4):
                            c = half * 8 + b4 * 4 + c4
                            nc.tensor.matmul(
                                o_ps[:, c4 * 128:(c4 + 1) * 128],
                                vT_sb[c][:, tg * 128:(tg + 1) * 128],
                                ident_b[:], is_transpose=True,
                                start=True, stop=True,
                                skip_group_check=True)
                        nc.scalar.copy(o_row[:, b4 * 512:(b4 + 1) * 512],
                                       o_ps[:])
                    trow = mega * MEGA_T + tg * 128
                    nc.sync.dma_start(
                        out_d[trow:trow + 128,
                              half * 1024:(half + 1) * 1024],
                        o_row[:])

    nc.compile()
    return nc


_CACHED = None


def _get_program():
    global _CACHED
    if _CACHED is None:
        _CACHED = build_program()
    return _CACHED


def make_in_maps(x, input_ids, tables, key_w, value_w, conv_w):
    x = np.asarray(x, np.float32)
    input_ids = np.asarray(input_ids, np.int32)
    tables = np.asarray(tables, np.float32)
    key_w = np.asarray(key_w, np.float32)
    value_w = np.asarray(value_w, np.float32)
    conv_w = np.asarray(conv_w, np.float32)

    wkT = np.ascontiguousarray(key_w.T).astype(ml_dtypes.bfloat16)
    wvT = np.ascontiguousarray(value_w.T).astype(ml_dtypes.bfloat16)
    conv_pp = np.zeros((128, N_DCH * KERNEL), np.float32)
    for c in range(N_DCH):
        for i in range(KERNEL):
            conv_pp[:, c * KERNEL + i] = conv_w[c * 128:(c + 1) * 128, 0, i]

    in_maps = []
    for b in range(B):
        # s_j[(i*8+g), t'] = ids[g*256 + t' - (2 - j)], zero for negative
        ids_pad = np.concatenate([np.zeros(2, np.int32), input_ids[b]])
        sj_all = np.zeros((3, 128, 256), np.int32)
        for j in range(3):
            shifted = ids_pad[j:j + T].reshape(8, 256)  # [g, t']
            sj_all[j] = np.tile(shifted, (16, 1))
        in_maps.append({
            "sj_all": sj_all,
            "tables": tables,
            "xrow": np.ascontiguousarray(x[b]),
            "wkT": wkT,
            "wvT": wvT,
            "consts_pp": CONSTS_PP,
            "conv_pp": conv_pp,
        })
    return in_maps


def host_reference_hash(input_ids):
    """Exact gather indices, [B, NT, T] int64 (for testing)."""
    ids = np.asarray(input_ids, np.int64)
    Bx, Tx = ids.shape
    out = np.zeros((Bx, NT, Tx), np.int64)
    ids_pad = np.concatenate([np.zeros((Bx, 2), np.int64), ids], axis=1)
    for i in range(NT):
        p = PRIMES[i]
        acc = np.zeros((Bx, Tx), np.int64)
        for j in range(3):
            s = ids_pad[:, j:j + Tx]
            acc = (acc + (s * W_UNI[i][j]) % p) % p
        out[:, i, :] = acc + int(OFFSETS[i])
    return out


def kernel(x, input_ids, tables, key_w, value_w, conv_w):
    nc = _get_program()
    in_maps = make_in_maps(x, input_ids, tables, key_w, value_w, conv_w)
    res = bass_utils.run_bass_kernel_spmd(nc, in_maps, core_ids=list(range(B)))
    out = np.stack([res.results[b]["out"] for b in range(B)], axis=0)
    return out.astype(np.float32)
